# revision 23
# baseline (speedup 1.0000x reference)
"""Trainium2 Bass kernel for nn_PointTransformerLayer_59674275611307.

Mathematical simplification: in the reference, the attention logits `w` are
broadcast identically across the NSAMPLE axis before the softmax.  Softmax
over identical values is exactly uniform (1/16 each), and the weights sum to
exactly 1, so the grouped weighted sum of values collapses to the values
themselves:

    out = (xv_g * attn).sum(axis=1) == xv == x @ Wv + bv

(verified: rel err ~2e-7 vs the full reference).  Everything else — the q/k
projections, the position MLP, both BN+MLP stacks and the softmax — cancels
out of the output entirely.  The kernel computes the single
(50000,64)@(64,64) matmul + bias, data-parallel over points across 8 cores.

Numeric scheme (avoids the fp32 weight-load wall on the PE): split both x
and Wv into fp16 (hi, scaled-lo) planes:

    x  = hi + 2^-11 * lo_s + O(2^-22)        (lo_s = f16((x - hi) * 2048))
    Wv = Wvb + 2^-11 * Wvr_s + O(2^-22)

    x @ Wv ~= hi @ Wvb  +  2^-11 * (lo_s @ Wvb + hi @ Wvr_s)

Both planes are in fp16 normal range (no subnormals), all products are
exact in the PE's fp32 accumulation, and the dropped lo_s@Wvr_s term is
~2^-22 relative — fp32-level accuracy overall (measured ~5e-7).

Device strategy (per core, 6400 rows after padding 50000 -> 51200):
  - host packs one dram tensor "xkw" [128, 128 + 6400] fp16: a 128-col
    header (Wvb stacked twice; Wvr_s) followed by the x planes transposed:
    partition k in 0:64 = hi[k, row n], partition 64+k = lo_s[k, row n],
    column 128+n.  The contraction dim k is the SBUF partition dim (the PE
    contracts over partitions) with zero on-device transposes and fully
    contiguous 128-partition DMAs.
  - weights are the PE-stationary operand (tiny, reloaded from SBUF in
    ~50ns), data is the moving operand at 1 cycle/row — no per-chunk
    weight-load wall.  Output is produced transposed (out.T[c, n] in PSUM);
    the host untransposes during the gather.
  - 16 batches of 400 rows; batches 2i/2i+1 write the top/bottom 64
    partitions of shared PSUM banks (concurrent via PE column-groups), one
    main + one correction bank per pair; a single DVE op per pair fuses
    (corr * 2^-11 + main) while copying PSUM -> SBUF.
  - dummy bf16 matmuls warm the PE HAM clock gate during the input DMAs.
"""

import numpy as np

N = 50000
C = 64
NCORES = 8
ROWS_PER_CORE = 6400
N_PAD = NCORES * ROWS_PER_CORE  # 51200
BATCH = 400                   # rows per matmul (moving free dim)
NBATCH = ROWS_PER_CORE // BATCH  # 16
NPAIR = NBATCH // 2           # 8 (top/bottom share a psum bank pair)
HDR = 128                     # header cols: Wvb (64) + Wvr_s (64)
LO_SCALE = 2048.0             # 2^11
# payload pairs per load piece (piece 0 also carries the header)
PIECE_PAIRS = [2, 2, 2, 2]
N_WARMUP = 8   # dummy bf16 matmuls to engage the PE HAM clock during DMA-in

TRACE = False          # test.py sets True to collect an NTFF profile
LAST_RESULT = None     # BassKernelResults of the last run (for test.py)

_cache = {}


def _get_compiled():
    if "nc" in _cache:
        return _cache["nc"]

    import concourse.mybir as mybir
    import concourse.tile as tile
    from concourse import bacc
    from concourse.alu_op_type import AluOpType
    from concourse.bass import MemorySpace

    f32 = mybir.dt.float32
    f16 = mybir.dt.float16
    bf16 = mybir.dt.bfloat16
    nc = bacc.Bacc("TRN2", target_bir_lowering=False, debug=False,
                   num_devices=NCORES)

    xkw_d = nc.dram_tensor("xkw", [128, HDR + ROWS_PER_CORE], f16,
                           kind="ExternalInput")
    out_d = nc.dram_tensor("out", [128, NBATCH // 2 * BATCH], f32,
                           kind="ExternalOutput")

    with tile.TileContext(nc) as tc:
        with (
            tc.tile_pool(name="const", bufs=1) as constp,
            tc.tile_pool(name="xt", bufs=1) as xtp,
            tc.tile_pool(name="outp", bufs=1) as outp,
            tc.tile_pool(name="ps", bufs=3, space=MemorySpace.PSUM) as psp,
        ):
            # PE warmup: dummy bf16 matmuls gated only on a DVE memset, so
            # they run during the input-DMA window and the HAM clock gate
            # reaches 8/8 before the real matmul stream starts.
            scr = constp.tile([128, 384], bf16, tag="scr")
            nc.vector.memset(scr[:], 0.0)
            ps_w = psp.tile([128, 512], f32, tag="warm", bufs=1)
            for _ in range(N_WARMUP):
                nc.tensor.matmul(ps_w[:, :384], scr[:, :128], scr[:],
                                 start=True, stop=True)

            # Input DMAs: piece 0 (header + first pairs) on sync, then
            # alternate sync/scalar so both HWDGE rings pipeline.
            xt_tiles = []   # (tile, first_batch, payload_col_offset)
            col = 0
            b0 = 0
            for i, npair in enumerate(PIECE_PAIRS):
                w = npair * 2 * BATCH + (HDR if i == 0 else 0)
                t_ = xtp.tile([128, w], f16, tag=f"xt{i}", name=f"xt_sb{i}")
                eng = nc.sync if i % 2 == 0 else nc.scalar
                eng.dma_start(t_[:], xkw_d.ap()[:, col:col + w])
                xt_tiles.append((t_, b0, HDR if i == 0 else 0))
                col += w
                b0 += npair * 2
            # header col 0:64  rows 0:64   = Wvb * 2^11  (main, up-scaled)
            # header col 64:128 rows 0:64  = Wvr_s       (stacked corr wts)
            #                  rows 64:128 = Wvb
            # PSUM accumulates 2^11*(hi@Wvb + lo@Wvb + hi@Wvr); the DVE
            # copy scales by 2^-11.  All fp16 values stay in normal range.
            wv_main = xt_tiles[0][0][0:64, 0:C]      # [64, 64] f16
            wv_corr = xt_tiles[0][0][:, C:2 * C]     # [128, 64] f16

            def batch_cols(b):
                """SBUF AP [128, BATCH] of batch b's moving data."""
                for t_, first, off in xt_tiles:
                    nb = (t_.shape[1] - off) // BATCH
                    if first <= b < first + nb:
                        lo = off + (b - first) * BATCH
                        return t_[:, lo:lo + BATCH]
                raise AssertionError(b)

            out_sb = outp.tile([128, NPAIR * BATCH], f32, tag="out")

            for i in range(NPAIR):
                ps = psp.tile([128, BATCH], f32, tag="acc")
                for h in (0, 1):            # top / bottom half (col groups)
                    b = 2 * i + h
                    rhs = batch_cols(b)
                    sl = slice(64 * h, 64 * h + 64)
                    # main: 2^11 * hi @ Wvb  (contract over partitions 0:64)
                    nc.tensor.matmul(ps[sl, :], wv_main, rhs[0:64, :],
                                     start=True, stop=False)
                    # corr: 2^11*(hi@Wvr + lo@Wvb), accumulated
                    nc.tensor.matmul(ps[sl, :], wv_corr, rhs,
                                     start=False, stop=True)
                # out = psum * 2^-11   (PSUM -> SBUF, one DVE op per pair)
                nc.vector.tensor_scalar_mul(
                    out_sb[:, i * BATCH:(i + 1) * BATCH], ps[:, :],
                    1.0 / LO_SCALE)

            # stores: one per 2 pairs (800 f32 cols = 400KB)
            for s in range(NPAIR // 2):
                lo = s * 2 * BATCH
                nc.sync.dma_start(out_d.ap()[:, lo:lo + 2 * BATCH],
                                  out_sb[:, lo:lo + 2 * BATCH])

    nc.compile()
    _cache["nc"] = nc
    return nc


def pack_inputs(x, Wv):
    """Build the per-core [128, 6528] fp16 xkw arrays (header + planes)."""
    x_pad = np.zeros((N_PAD, C), np.float32)
    x_pad[:N] = x
    xs = x_pad.reshape(NCORES, ROWS_PER_CORE, C)
    hi = xs.astype(np.float16)
    lo_s = ((xs - hi.astype(np.float32)) * LO_SCALE).astype(np.float16)
    xk = np.concatenate([hi.transpose(0, 2, 1), lo_s.transpose(0, 2, 1)],
                        axis=1)                     # [8, 128, 6400] f16
    Wvb = Wv.astype(np.float16)
    Wvr_s = ((Wv - Wvb.astype(np.float32)) * LO_SCALE).astype(np.float16)
    hdr = np.zeros((NCORES, 128, HDR), np.float16)
    hdr[:, 0:64, 0:C] = (Wvb.astype(np.float32) * LO_SCALE).astype(
        np.float16)   # exact power-of-2 scale
    hdr[:, 0:64, C:2 * C] = Wvr_s
    hdr[:, 64:128, C:2 * C] = Wvb
    return np.ascontiguousarray(np.concatenate([hdr, xk], axis=2))


def unpack_output(res_list, bv):
    """[128, 3200] per-core device outputs -> (N, 64), plus bias."""
    outs = []
    for r in res_list:
        o = r["out"].reshape(2, 64, NPAIR, BATCH)
        # out_dev[h*64+c, i*400+j] = out(row (2i+h)*400+j, chan c)
        outs.append(o.transpose(2, 0, 3, 1).reshape(ROWS_PER_CORE, C))
    out = np.concatenate(outs, axis=0)[:N]
    if np.any(bv):
        out = out + bv[None, :].astype(np.float32)
    return np.ascontiguousarray(out.astype(np.float32))


def kernel(**inputs):
    global LAST_RESULT
    x = np.asarray(inputs["x"], dtype=np.float32)
    Wv = np.asarray(inputs["Wv"], dtype=np.float32)
    bv = np.asarray(inputs["bv"], dtype=np.float32)

    nc = _get_compiled()
    xkw = pack_inputs(x, Wv)

    from concourse.bass_utils import run_bass_kernel_spmd
    in_maps = [{"xkw": xkw[i]} for i in range(NCORES)]
    res = run_bass_kernel_spmd(nc, in_maps, list(range(NCORES)),
                               trace=TRACE)
    LAST_RESULT = res
    return unpack_output(res.results, bv)
